# revision 37
# baseline (speedup 1.0000x reference)
"""Trainium2 Bass kernel for the CapsuleLayer dynamic-routing module.

Strategy (8 NeuronCores, zero collectives):
  The [J,N] agreement update needs a mean over the full batch. Instead of
  data-parallel + AllReduce (first-execution ncfw staging costs ~25-45us
  per collective op and gates the routing loop), every core redundantly
  computes the FULL-batch s-pass and a-pass for routing iterations 0 and
  1, so the b_ij update is computed locally and identically on all cores.
  Only the last iteration's s-pass/squash runs on the core's own 32-row
  batch slice to produce its output shard. Extra cost: 2x matmul work on
  iters 0/1 plus ~9.4 MB of replicated x layouts streamed during iter-0
  compute. No inter-core sync at all.

  Host pre-lays-out (not part of measured HW time):
      wb   [128, (i=8, jt=9, n=10, u=16)]  bf16  -- W[j,n,u,i], j = 128*jt + p
      x2f  [128, (i, jt, h=2, c=128)]      bf16  -- x[b,i,j]: b = 128h + c
      xnf  [128, (i, h=2, j=1152)]         bf16  -- partitions = b % 128
      x2   [128, (t=18, c=128)]            bf16  -- core-local slice,
                                                    t=(i%2)*9+jt, c=4b+i//2
  Per routing iteration (full-batch, iters 0/1):
      s-pass:  144 accumulated PE matmuls (k=128 j-partitions, two b-halves
               into one PSUM tile [128, 2, 160]); lhsT = x2f slices,
               rhs = A slices (A = W*c).
      squash on [128, 2, (n,u)] with exact reference semantics; 1/B mean
               scale folded into the bf16 v used by the a-pass.
      a-pass:  C[(i,j), nu] = x^T v (PE, k=2x128 accumulated); z = Wb.*C
               and pairwise u-fold + i-reduce on DVE; b += agreement.
      c-pass:  cexp = exp(b); D[n] via ones-matmul; A_i = Wb_i * cexp_u.
  Last iteration: 72 local matmuls (lhsT = core's x2 slices), local squash,
  DMA v [32, 160] out.
"""

import numpy as np

B, I, J, N, U = 256, 8, 1152, 10, 16
NU = N * U            # 160
ITERS = 3
NCORES = 8
BL = B // NCORES      # 32
JT = 9                # 1152 / 128
H = 2                 # 256 / 128 batch halves

_CACHE = {}


def _build_nc():
    import concourse.bass as bass
    import concourse.bacc as bacc
    import concourse.tile as tile
    from concourse import mybir

    f32 = mybir.dt.float32
    bf16 = mybir.dt.bfloat16
    fp8 = mybir.dt.float8e4
    AL = mybir.AluOpType
    AF = mybir.ActivationFunctionType
    AX = mybir.AxisListType

    nc = bacc.Bacc("TRN2", target_bir_lowering=False, debug=False,
                   num_devices=NCORES)
    wb_d = nc.dram_tensor("wb", [128, I * JT * NU], bf16, kind="ExternalInput").ap()
    x2f_d = nc.dram_tensor("x2f", [128, I * JT * H * 128], fp8,
                           kind="ExternalInput").ap()
    xnf_d = nc.dram_tensor("xnf", [128, I * H * J], bf16,
                           kind="ExternalInput").ap()
    x2_d = nc.dram_tensor("x2", [128, 18 * 128], bf16, kind="ExternalInput").ap()
    v_d = nc.dram_tensor("v", [BL, NU], f32, kind="ExternalOutput").ap()

    with tile.TileContext(nc) as tc:
        with (
            tc.tile_pool(name="big", bufs=1) as big,
            tc.tile_pool(name="abp", bufs=1) as abp,
            tc.tile_pool(name="small", bufs=2) as small,
            tc.tile_pool(name="pers", bufs=1) as pers,
            tc.tile_pool(name="ps_s", bufs=1, space="PSUM") as ps_s,
            tc.tile_pool(name="ps_c", bufs=1, space="PSUM") as ps_c,
        ):
            # ---------------- load inputs ----------------
            # Interleave x2f_i / Wb_i per i so the iter-0 s-pass can start
            # as soon as the first (x2f_0, Wb_0) pair lands.
            x2f_v = x2f_d.rearrange("p (i jt h c) -> p i jt h c",
                                    i=I, jt=JT, h=H)
            wb_v = wb_d.rearrange("p (i jt n u) -> p i jt n u",
                                  i=I, jt=JT, n=N, u=U)
            # fp8 halves the x2f HBM traffic and speeds the 128-col weight
            # loads; the s-pass runs mixed fp8 lhsT x bf16 rhs. Simulated
            # end-to-end rel-err impact: 3.2e-3 -> 3.4e-3.
            X2f = big.tile([128, I, JT, H, 128], fp8)
            Wb = big.tile([128, I, JT, N, U], bf16)
            for i in range(I):
                nc.sync.dma_start(out=X2f[:, i], in_=x2f_v[:, i])
                nc.sync.dma_start(out=Wb[:, i], in_=wb_v[:, i])
            xnf_v = xnf_d.rearrange("p (i h j) -> p i h j", i=I, h=H)
            XNf = big.tile([128, I, H, J], bf16)
            for i in range(I):
                nc.sync.dma_start(out=XNf[:, i], in_=xnf_v[:, i])
            X2 = big.tile([128, 18, 128], bf16)
            nc.sync.dma_start(out=X2, in_=x2_d.rearrange(
                "p (t c) -> p t c", t=18))

            ones = pers.tile([128, 128], bf16)
            nc.vector.memset(ones, 1.0)
            bmat = pers.tile([128, JT * N], f32)          # b[j, n] as [p, (jt, n)]
            nc.vector.memset(bmat, 0.0)

            X2v = X2.rearrange("p t (b ih) -> p t ih b", ih=4)

            for it in range(ITERS):
                first = it == 0
                last = it == ITERS - 1

                # ------------ c-pass: A and Dinv ------------
                if first:
                    As = Wb
                    Dinv = small.tile([128, N], f32, tag="dinv")
                    nc.vector.memset(Dinv, 1.0 / J)
                else:
                    cexp_b = small.tile([128, JT * N], bf16, tag="cexpb")
                    nc.scalar.activation(out=cexp_b, in_=bmat, func=AF.Exp)
                    # D[n] = sum_{p,jt} cexp -> ones-matmul (replicated over
                    # all 128 partitions) + jt-reduce
                    psd = ps_c.tile([128, 3, 512], f32, tag="c0")
                    nc.tensor.matmul(psd[:, 0, 0:JT * N], lhsT=ones,
                                     rhs=cexp_b, start=True, stop=True)
                    D32 = small.tile([128, N], f32, tag="d32")
                    nc.vector.tensor_reduce(
                        out=D32,
                        in_=psd[:, 0, 0:JT * N].rearrange(
                            "q (jt n) -> q n jt", jt=JT),
                        axis=AX.X, op=AL.add)
                    Dinv = small.tile([128, N], f32, tag="dinv")
                    nc.vector.reciprocal(out=Dinv, in_=D32)
                    # cexp_u[p, jt, n, u] = cexp_b[p, jt, n]  (broadcast u)
                    cexp_u = small.tile([128, JT, N, U], bf16, tag="cexpu")
                    nc.vector.tensor_copy(
                        out=cexp_u,
                        in_=cexp_b.rearrange("p (jt n) -> p jt n", jt=JT)
                        .unsqueeze(3).broadcast_to([128, JT, N, U]))
                    cexp_flat = cexp_u.rearrange("p jt n u -> p (jt n u)")
                    As = abp.tile([128, I, JT, N, U], bf16, tag="Aall")
                    for i in range(I):
                        nc.vector.tensor_tensor(
                            out=As[:, i].rearrange("p jt n u -> p (jt n u)"),
                            in0=Wb[:, i].rearrange("p jt n u -> p (jt n u)"),
                            in1=cexp_flat, op=AL.mult)

                if last:
                    # ------------ local s-pass: 72 matmuls ------------
                    pss = ps_s.tile([BL, NU], f32, tag="pss")
                    k = 0
                    for i in range(I):
                        il, ih = i % 2, i // 2
                        for jt in range(JT):
                            nc.tensor.matmul(
                                pss,
                                lhsT=X2v[:, il * JT + jt, ih, :],
                                rhs=As[:, i, jt, :, :],
                                start=(k == 0), stop=(k == 71))
                            k += 1
                    # ------------ local squash + output ------------
                    s_sc = small.tile([BL, N, U], f32, tag="ssc")
                    nc.vector.tensor_tensor(
                        out=s_sc,
                        in0=pss.rearrange("b (n u) -> b n u", n=N),
                        in1=Dinv[0:BL].unsqueeze(2).broadcast_to([BL, N, U]),
                        op=AL.mult)
                    sq = small.tile([BL, N, U], f32, tag="sq")
                    nc.vector.tensor_tensor(out=sq, in0=s_sc, in1=s_sc,
                                            op=AL.mult)
                    mag = small.tile([BL, U], f32, tag="mag")
                    nc.vector.tensor_reduce(
                        out=mag, in_=sq.rearrange("b n u -> b u n"),
                        axis=AX.X, op=AL.add)
                    sqrtm = small.tile([BL, U], f32, tag="sqrtm")
                    nc.scalar.activation(out=sqrtm, in_=mag, func=AF.Sqrt)
                    onep = small.tile([BL, U], f32, tag="onep")
                    nc.vector.tensor_scalar_add(out=onep, in0=mag, scalar1=1.0)
                    rec = small.tile([BL, U], f32, tag="rec")
                    nc.vector.reciprocal(out=rec, in_=onep)
                    g = small.tile([BL, U], f32, tag="g")
                    nc.vector.tensor_tensor(out=g, in0=sqrtm, in1=rec,
                                            op=AL.mult)
                    v_f32 = small.tile([BL, N, U], f32, tag="vf32")
                    nc.vector.tensor_tensor(
                        out=v_f32, in0=s_sc,
                        in1=g.unsqueeze(1).broadcast_to([BL, N, U]),
                        op=AL.mult)
                    nc.sync.dma_start(
                        out=v_d, in_=v_f32.rearrange("b n u -> b (n u)"))
                    break

                # ------------ full s-pass: 144 accumulated matmuls ------------
                PSS = ps_s.tile([128, H, NU], f32)
                for i in range(I):
                    for jt in range(JT):
                        for h in range(H):
                            nc.tensor.matmul(
                                PSS[:, h, :],
                                lhsT=X2f[:, i, jt, h, :],
                                rhs=As[:, i, jt, :, :],
                                start=(i == 0 and jt == 0),
                                stop=(i == I - 1 and jt == JT - 1))

                # ------------ full squash ------------
                s_sc = small.tile([128, H, N, U], f32, tag="fssc")
                nc.vector.tensor_tensor(
                    out=s_sc,
                    in0=PSS.rearrange("q h (n u) -> q h n u", n=N),
                    in1=Dinv.unsqueeze(1).unsqueeze(3)
                        .broadcast_to([128, H, N, U]),
                    op=AL.mult)
                sq = small.tile([128, H, N, U], f32, tag="fsq")
                nc.vector.tensor_tensor(out=sq, in0=s_sc, in1=s_sc,
                                        op=AL.mult)
                mag = small.tile([128, H, U], f32, tag="fmag")
                nc.vector.tensor_reduce(
                    out=mag, in_=sq.rearrange("q h n u -> q h u n"),
                    axis=AX.X, op=AL.add)
                sqrtm = small.tile([128, H, U], f32, tag="fsqrtm")
                nc.scalar.activation(out=sqrtm, in_=mag, func=AF.Sqrt)
                onep = small.tile([128, H, U], f32, tag="fonep")
                nc.vector.tensor_scalar_add(out=onep, in0=mag, scalar1=1.0)
                rec = small.tile([128, H, U], f32, tag="frec")
                nc.vector.reciprocal(out=rec, in_=onep)
                # fold the 1/B mean scale into g; emit bf16 v directly
                g = small.tile([128, H, U], f32, tag="fg")
                nc.vector.scalar_tensor_tensor(
                    out=g, in0=sqrtm, scalar=1.0 / B, in1=rec,
                    op0=AL.mult, op1=AL.mult)
                vb16 = small.tile([128, H, N, U], bf16, tag="fvb16")
                nc.vector.tensor_tensor(
                    out=vb16, in0=s_sc,
                    in1=g.unsqueeze(2).broadcast_to([128, H, N, U]),
                    op=AL.mult)

                # ------------ full a-pass ------------
                # Per i: C matmuls (k = 2x128 accumulated over b) into a
                # 3-bank PSUM tile -> ONE ACT drain. The DVE product + u-fold
                # tree runs FUSED over i-pairs (0,1),(2,3),(4,5) to halve the
                # per-op dispatch overhead; i=6,7 stay single so the serial
                # tail chain after the last matmul is short. The i-reduction
                # over 0..5 is issued early, off the tail.
                z2 = small.tile([128, I, JT, N], bf16, tag="z2")
                for grp in ((0, 1), (2, 3), (4, 5)):
                    g = len(grp)
                    Cb = small.tile([128, g, JT, N, U], bf16, tag="Cb2")
                    for gi, i in enumerate(grp):
                        psc = ps_c.tile([128, 3, 512], f32, tag=f"c{i % 2}")
                        for jt in range(JT):
                            for h in range(H):
                                nc.tensor.matmul(
                                    psc[:, jt // 3,
                                        (jt % 3) * NU:(jt % 3 + 1) * NU],
                                    lhsT=XNf[:, i, h,
                                             jt * 128:(jt + 1) * 128],
                                    rhs=vb16[:, h, :, :],
                                    start=(h == 0), stop=(h == H - 1))
                        nc.scalar.copy(
                            out=Cb[:, gi].rearrange("p jt n u -> p (jt n u)"),
                            in_=psc[:, :, 0:3 * NU])
                    i0 = grp[0]
                    zt = small.tile([128, g, JT, N, U], bf16, tag="zt2")
                    nc.vector.tensor_tensor(
                        out=zt.rearrange("p g jt n u -> p (g jt n u)"),
                        in0=Wb[:, i0:i0 + g].rearrange(
                            "p g jt n u -> p (g jt n u)"),
                        in1=Cb.rearrange("p g jt n u -> p (g jt n u)"),
                        op=AL.mult)
                    t8 = small.tile([128, g, JT, N, 8], bf16, tag="t82")
                    nc.vector.tensor_tensor(out=t8, in0=zt[:, :, :, :, 0:8],
                                            in1=zt[:, :, :, :, 8:16],
                                            op=AL.add)
                    t4 = small.tile([128, g, JT, N, 4], bf16, tag="t42")
                    nc.vector.tensor_tensor(out=t4, in0=t8[:, :, :, :, 0:4],
                                            in1=t8[:, :, :, :, 4:8],
                                            op=AL.add)
                    t2 = small.tile([128, g, JT, N, 2], bf16, tag="t22")
                    nc.vector.tensor_tensor(out=t2, in0=t4[:, :, :, :, 0:2],
                                            in1=t4[:, :, :, :, 2:4],
                                            op=AL.add)
                    nc.vector.tensor_tensor(out=z2[:, i0:i0 + g, :, :],
                                            in0=t2[:, :, :, :, 0],
                                            in1=t2[:, :, :, :, 1], op=AL.add)
                    if grp == (4, 5):
                        # fold i=0..5 of the i-reduction off the tail; on
                        # GpSimd so it doesn't lengthen the DVE fold stream
                        alo = small.tile([128, JT * N], bf16, tag="alo")
                        with nc.allow_low_precision(
                                reason="agreement accumulate; bf16 rounding "
                                       "is within tolerance"):
                            nc.vector.tensor_reduce(
                                out=alo,
                                in_=z2[:, 0:6].rearrange(
                                    "p i jt n -> p (jt n) i"),
                                axis=AX.X, op=AL.add)
                # i = 6, 7: per-jt-group drains and fold chains so the serial
                # DVE tail after the very last C matmul is ~1/3 length.
                for i in (6, 7):
                    Cb = small.tile([128, JT, N, U], bf16, tag="Cb1")
                    psc = ps_c.tile([128, 3, 512], f32, tag=f"c{i % 2}")
                    for jt in range(JT):
                        for h in range(H):
                            nc.tensor.matmul(
                                psc[:, jt // 3,
                                    (jt % 3) * NU:(jt % 3 + 1) * NU],
                                lhsT=XNf[:, i, h, jt * 128:(jt + 1) * 128],
                                rhs=vb16[:, h, :, :],
                                start=(h == 0), stop=(h == H - 1))
                    for jg in range(3):
                        sl = slice(jg * 3, (jg + 1) * 3)
                        nc.scalar.copy(
                            out=Cb[:, sl].rearrange("p jt n u -> p (jt n u)"),
                            in_=psc[:, jg, 0:3 * NU])
                        zt = small.tile([128, 3, N, U], bf16, tag="zt1")
                        nc.vector.tensor_tensor(
                            out=zt.rearrange("p jt n u -> p (jt n u)"),
                            in0=Wb[:, i, sl].rearrange(
                                "p jt n u -> p (jt n u)"),
                            in1=Cb[:, sl].rearrange("p jt n u -> p (jt n u)"),
                            op=AL.mult)
                        t8 = small.tile([128, 3, N, 8], bf16, tag="t81")
                        nc.vector.tensor_tensor(out=t8, in0=zt[:, :, :, 0:8],
                                                in1=zt[:, :, :, 8:16],
                                                op=AL.add)
                        t4 = small.tile([128, 3, N, 4], bf16, tag="t41")
                        nc.vector.tensor_tensor(out=t4, in0=t8[:, :, :, 0:4],
                                                in1=t8[:, :, :, 4:8],
                                                op=AL.add)
                        t2 = small.tile([128, 3, N, 2], bf16, tag="t21")
                        nc.vector.tensor_tensor(out=t2, in0=t4[:, :, :, 0:2],
                                                in1=t4[:, :, :, 2:4],
                                                op=AL.add)
                        nc.vector.tensor_tensor(out=z2[:, i, sl, :],
                                                in0=t2[:, :, :, 0],
                                                in1=t2[:, :, :, 1],
                                                op=AL.add)
                a67 = small.tile([128, JT * N], bf16, tag="a67")
                nc.vector.tensor_tensor(
                    out=a67,
                    in0=z2[:, 6].rearrange("p jt n -> p (jt n)"),
                    in1=z2[:, 7].rearrange("p jt n -> p (jt n)"), op=AL.add)
                apart = small.tile([128, JT * N], bf16, tag="apart")
                nc.vector.tensor_tensor(out=apart, in0=alo, in1=a67,
                                        op=AL.add)
                nc.vector.tensor_tensor(out=bmat, in0=bmat, in1=apart,
                                        op=AL.add)

    nc.compile()
    return nc


def _prep_inputs(x_full, W):
    """Host-side relayout. x_full: [B, I, J] f32, W: [J, N, U, I] f32."""
    import ml_dtypes
    bf = ml_dtypes.bfloat16
    # Wb[p, i, jt, n, u] = W[128*jt+p, n, u, i]
    Wb = np.ascontiguousarray(
        W.reshape(JT, 128, N, U, I).transpose(1, 4, 0, 2, 3)
    ).reshape(128, I * JT * N * U).astype(bf)
    f8 = ml_dtypes.float8_e4m3fn
    # x2f[p, i, jt, h, c] = x[128h+c, i, 128jt+p]
    X2F = np.ascontiguousarray(
        x_full.reshape(H, 128, I, JT, 128).transpose(4, 2, 3, 0, 1)
    ).reshape(128, I * JT * H * 128).astype(f8)
    # xnf[c, i, h, j] = x[128h+c, i, j]
    XNF = np.ascontiguousarray(
        x_full.reshape(H, 128, I, J).transpose(1, 2, 0, 3)
    ).reshape(128, I * H * J).astype(bf)
    in_maps = []
    for c in range(NCORES):
        xc = x_full[c * BL:(c + 1) * BL]                   # [32, 8, 1152]
        # x128[4b+ih, il, j] = xc[b, 2*ih+il, j]
        x128 = xc.reshape(BL, 4, 2, J).reshape(128, 2, J)
        # X2[p, t=(il*9+jt), c] = x128[c, il, 128*jt+p]
        X2 = np.ascontiguousarray(
            x128.reshape(128, 2, JT, 128).transpose(3, 1, 2, 0)
        ).reshape(128, 18 * 128).astype(bf)
        in_maps.append({"wb": Wb, "x2f": X2F, "xnf": XNF, "x2": X2})
    return in_maps


def kernel(x, W):
    """x: [256, 8, 1152] f32; W: [1152, 10, 16, 8] f32 ->
    v: [256, 10, 16, 1] f32."""
    from concourse.bass_utils import run_bass_kernel_spmd

    x = np.asarray(x, dtype=np.float32)
    W = np.asarray(W, dtype=np.float32)
    if "nc" not in _CACHE:
        _CACHE["nc"] = _build_nc()
    nc = _CACHE["nc"]
    in_maps = _prep_inputs(x, W)
    res = run_bass_kernel_spmd(nc, in_maps, core_ids=list(range(NCORES)))
    out = np.concatenate([r["v"] for r in res.results], axis=0)
    return out.reshape(B, N, U, 1).astype(np.float32)


if __name__ == "__main__":
    rng = np.random.default_rng(0)
    x = rng.standard_normal((B, I, J), dtype=np.float32)
    W = rng.standard_normal((J, N, U, I), dtype=np.float32)
    got = kernel(x, W)
    # numpy reference for a self-contained smoke test
    u_hat = np.einsum('jnui,bij->bjnu', W, x)
    b = np.zeros((J, N), dtype=np.float32)
    for _ in range(ITERS):
        e = np.exp(b - b.max(axis=0, keepdims=True))
        c = e / e.sum(axis=0, keepdims=True)
        s = np.einsum('jn,bjnu->bnu', c, u_hat)
        mag = np.sum(s * s, axis=1, keepdims=True)
        v = (mag / (1.0 + mag)) * (s / np.sqrt(mag))
        b = b + np.einsum('bjnu,bnu->jn', u_hat, v) / B
    exp = v[..., None]
    rel = np.linalg.norm(got - exp) / np.linalg.norm(exp)
    print("rel_fro:", rel)


# revision 42
# speedup vs baseline: 1.0172x; 1.0172x over previous
"""Trainium2 Bass kernel for the CapsuleLayer dynamic-routing module.

Strategy (8 NeuronCores, zero collectives):
  The [J,N] agreement update needs a mean over the full batch. Instead of
  data-parallel + AllReduce (first-execution ncfw staging costs ~25-45us
  per collective op and gates the routing loop), every core redundantly
  computes the FULL-batch s-pass and a-pass for routing iterations 0 and
  1, so the b_ij update is computed locally and identically on all cores.
  Only the last iteration's s-pass/squash runs on the core's own 32-row
  batch slice to produce its output shard. Extra cost: 2x matmul work on
  iters 0/1 plus ~9.4 MB of replicated x layouts streamed during iter-0
  compute. No inter-core sync at all.

  Host pre-lays-out (not part of measured HW time):
      wb   [128, (i=8, jt=9, n=10, u=16)]  bf16  -- W[j,n,u,i], j = 128*jt + p
      x2f  [128, (i, jt, h=2, c=128)]      bf16  -- x[b,i,j]: b = 128h + c
      xnf  [128, (i, h=2, j=1152)]         bf16  -- partitions = b % 128
      x2   [128, (t=18, c=128)]            bf16  -- core-local slice,
                                                    t=(i%2)*9+jt, c=4b+i//2
  Per routing iteration (full-batch, iters 0/1):
      s-pass:  144 accumulated PE matmuls (k=128 j-partitions, two b-halves
               into one PSUM tile [128, 2, 160]); lhsT = x2f slices,
               rhs = A slices (A = W*c).
      squash on [128, 2, (n,u)] with exact reference semantics; 1/B mean
               scale folded into the bf16 v used by the a-pass.
      a-pass:  C[(i,j), nu] = x^T v (PE, k=2x128 accumulated); z = Wb.*C
               and pairwise u-fold + i-reduce on DVE; b += agreement.
      c-pass:  cexp = exp(b); D[n] via ones-matmul; A_i = Wb_i * cexp_u.
  Last iteration: 72 local matmuls (lhsT = core's x2 slices), local squash,
  DMA v [32, 160] out.
"""

import numpy as np

B, I, J, N, U = 256, 8, 1152, 10, 16
NU = N * U            # 160
ITERS = 3
NCORES = 8
BL = B // NCORES      # 32
JT = 9                # 1152 / 128
H = 2                 # 256 / 128 batch halves

_CACHE = {}


def _build_nc():
    import concourse.bass as bass
    import concourse.bacc as bacc
    import concourse.tile as tile
    from concourse import mybir

    f32 = mybir.dt.float32
    bf16 = mybir.dt.bfloat16
    fp8 = mybir.dt.float8e4
    AL = mybir.AluOpType
    AF = mybir.ActivationFunctionType
    AX = mybir.AxisListType

    nc = bacc.Bacc("TRN2", target_bir_lowering=False, debug=False,
                   num_devices=NCORES)
    wb_d = nc.dram_tensor("wb", [128, I * JT * NU], bf16, kind="ExternalInput").ap()
    x2f_d = nc.dram_tensor("x2f", [128, I * JT * H * 128], fp8,
                           kind="ExternalInput").ap()
    xnf_d = nc.dram_tensor("xnf", [128, I * H * J], fp8,
                           kind="ExternalInput").ap()
    x2_d = nc.dram_tensor("x2", [128, 18 * 128], bf16, kind="ExternalInput").ap()
    v_d = nc.dram_tensor("v", [BL, NU], f32, kind="ExternalOutput").ap()

    with tile.TileContext(nc) as tc:
        with (
            tc.tile_pool(name="big", bufs=1) as big,
            tc.tile_pool(name="abp", bufs=1) as abp,
            tc.tile_pool(name="small", bufs=2) as small,
            tc.tile_pool(name="pers", bufs=1) as pers,
            tc.tile_pool(name="ps_s", bufs=1, space="PSUM") as ps_s,
            tc.tile_pool(name="ps_c", bufs=1, space="PSUM") as ps_c,
        ):
            # ---------------- load inputs ----------------
            # Interleave x2f_i / Wb_i per i so the iter-0 s-pass can start
            # as soon as the first (x2f_0, Wb_0) pair lands.
            x2f_v = x2f_d.rearrange("p (i jt h c) -> p i jt h c",
                                    i=I, jt=JT, h=H)
            wb_v = wb_d.rearrange("p (i jt n u) -> p i jt n u",
                                  i=I, jt=JT, n=N, u=U)
            # fp8 halves the x2f HBM traffic and speeds the 128-col weight
            # loads; the s-pass runs mixed fp8 lhsT x bf16 rhs. Simulated
            # end-to-end rel-err impact: 3.2e-3 -> 3.4e-3.
            X2f = big.tile([128, I, JT, H, 128], fp8)
            Wb = big.tile([128, I, JT, N, U], bf16)
            for i in range(I):
                nc.sync.dma_start(out=X2f[:, i], in_=x2f_v[:, i])
                nc.sync.dma_start(out=Wb[:, i], in_=wb_v[:, i])
            # fp8 x for the a-pass too (C matmuls run mixed fp8 x bf16 v);
            # only quantizing v breaks tolerance, x-side noise averages out.
            xnf_v = xnf_d.rearrange("p (i h j) -> p i h j", i=I, h=H)
            XNf = big.tile([128, I, H, J], fp8)
            for i in range(I):
                nc.sync.dma_start(out=XNf[:, i], in_=xnf_v[:, i])
            X2 = big.tile([128, 18, 128], bf16)
            nc.sync.dma_start(out=X2, in_=x2_d.rearrange(
                "p (t c) -> p t c", t=18))

            ones = pers.tile([128, 128], bf16)
            nc.vector.memset(ones, 1.0)
            bmat = pers.tile([128, JT * N], f32)          # b[j, n] as [p, (jt, n)]
            nc.vector.memset(bmat, 0.0)

            X2v = X2.rearrange("p t (b ih) -> p t ih b", ih=4)

            for it in range(ITERS):
                first = it == 0
                last = it == ITERS - 1

                # ------------ c-pass: A and Dinv ------------
                if first:
                    As = Wb
                    Dinv = small.tile([128, N], f32, tag="dinv")
                    nc.vector.memset(Dinv, 1.0 / J)
                else:
                    cexp_b = small.tile([128, JT * N], bf16, tag="cexpb")
                    nc.scalar.activation(out=cexp_b, in_=bmat, func=AF.Exp)
                    # D[n] = sum_{p,jt} cexp -> ones-matmul (replicated over
                    # all 128 partitions) + jt-reduce
                    psd = ps_c.tile([128, 3, 512], f32, tag="c0")
                    nc.tensor.matmul(psd[:, 0, 0:JT * N], lhsT=ones,
                                     rhs=cexp_b, start=True, stop=True)
                    D32 = small.tile([128, N], f32, tag="d32")
                    nc.vector.tensor_reduce(
                        out=D32,
                        in_=psd[:, 0, 0:JT * N].rearrange(
                            "q (jt n) -> q n jt", jt=JT),
                        axis=AX.X, op=AL.add)
                    Dinv = small.tile([128, N], f32, tag="dinv")
                    nc.vector.reciprocal(out=Dinv, in_=D32)
                    As = abp.tile([128, I, JT, N, U], bf16, tag="Aall")
                    # A_0 reads cexp via a broadcast AP so the first s-pass
                    # matmuls don't wait for the cexp_u materialization.
                    nc.vector.tensor_tensor(
                        out=As[:, 0],
                        in0=Wb[:, 0],
                        in1=cexp_b.rearrange("p (jt n) -> p jt n", jt=JT)
                        .unsqueeze(3).broadcast_to([128, JT, N, U]),
                        op=AL.mult)
                    # cexp_u[p, jt, n, u] = cexp_b[p, jt, n]  (broadcast u)
                    cexp_u = small.tile([128, JT, N, U], bf16, tag="cexpu")
                    nc.vector.tensor_copy(
                        out=cexp_u,
                        in_=cexp_b.rearrange("p (jt n) -> p jt n", jt=JT)
                        .unsqueeze(3).broadcast_to([128, JT, N, U]))
                    cexp_flat = cexp_u.rearrange("p jt n u -> p (jt n u)")
                    for i in range(1, I):
                        nc.vector.tensor_tensor(
                            out=As[:, i].rearrange("p jt n u -> p (jt n u)"),
                            in0=Wb[:, i].rearrange("p jt n u -> p (jt n u)"),
                            in1=cexp_flat, op=AL.mult)

                if last:
                    # ------------ local s-pass: 72 matmuls ------------
                    pss = ps_s.tile([BL, NU], f32, tag="pss")
                    k = 0
                    for i in range(I):
                        il, ih = i % 2, i // 2
                        for jt in range(JT):
                            nc.tensor.matmul(
                                pss,
                                lhsT=X2v[:, il * JT + jt, ih, :],
                                rhs=As[:, i, jt, :, :],
                                start=(k == 0), stop=(k == 71))
                            k += 1
                    # ------------ local squash + output ------------
                    s_sc = small.tile([BL, N, U], f32, tag="ssc")
                    nc.vector.tensor_tensor(
                        out=s_sc,
                        in0=pss.rearrange("b (n u) -> b n u", n=N),
                        in1=Dinv[0:BL].unsqueeze(2).broadcast_to([BL, N, U]),
                        op=AL.mult)
                    sq = small.tile([BL, N, U], f32, tag="sq")
                    nc.vector.tensor_tensor(out=sq, in0=s_sc, in1=s_sc,
                                            op=AL.mult)
                    mag = small.tile([BL, U], f32, tag="mag")
                    nc.vector.tensor_reduce(
                        out=mag, in_=sq.rearrange("b n u -> b u n"),
                        axis=AX.X, op=AL.add)
                    sqrtm = small.tile([BL, U], f32, tag="sqrtm")
                    nc.scalar.activation(out=sqrtm, in_=mag, func=AF.Sqrt)
                    onep = small.tile([BL, U], f32, tag="onep")
                    nc.vector.tensor_scalar_add(out=onep, in0=mag, scalar1=1.0)
                    rec = small.tile([BL, U], f32, tag="rec")
                    nc.vector.reciprocal(out=rec, in_=onep)
                    g = small.tile([BL, U], f32, tag="g")
                    nc.vector.tensor_tensor(out=g, in0=sqrtm, in1=rec,
                                            op=AL.mult)
                    v_f32 = small.tile([BL, N, U], f32, tag="vf32")
                    nc.vector.tensor_tensor(
                        out=v_f32, in0=s_sc,
                        in1=g.unsqueeze(1).broadcast_to([BL, N, U]),
                        op=AL.mult)
                    nc.sync.dma_start(
                        out=v_d, in_=v_f32.rearrange("b n u -> b (n u)"))
                    break

                # ------------ full s-pass: 144 accumulated matmuls ------------
                PSS = ps_s.tile([128, H, NU], f32)
                for i in range(I):
                    for jt in range(JT):
                        for h in range(H):
                            nc.tensor.matmul(
                                PSS[:, h, :],
                                lhsT=X2f[:, i, jt, h, :],
                                rhs=As[:, i, jt, :, :],
                                start=(i == 0 and jt == 0),
                                stop=(i == I - 1 and jt == JT - 1))

                # ------------ full squash ------------
                s_sc = small.tile([128, H, N, U], f32, tag="fssc")
                nc.vector.tensor_tensor(
                    out=s_sc,
                    in0=PSS.rearrange("q h (n u) -> q h n u", n=N),
                    in1=Dinv.unsqueeze(1).unsqueeze(3)
                        .broadcast_to([128, H, N, U]),
                    op=AL.mult)
                sq = small.tile([128, H, N, U], f32, tag="fsq")
                nc.vector.tensor_tensor(out=sq, in0=s_sc, in1=s_sc,
                                        op=AL.mult)
                mag = small.tile([128, H, U], f32, tag="fmag")
                nc.vector.tensor_reduce(
                    out=mag, in_=sq.rearrange("q h n u -> q h u n"),
                    axis=AX.X, op=AL.add)
                sqrtm = small.tile([128, H, U], f32, tag="fsqrtm")
                nc.scalar.activation(out=sqrtm, in_=mag, func=AF.Sqrt)
                onep = small.tile([128, H, U], f32, tag="fonep")
                nc.vector.tensor_scalar_add(out=onep, in0=mag, scalar1=1.0)
                rec = small.tile([128, H, U], f32, tag="frec")
                nc.vector.reciprocal(out=rec, in_=onep)
                # fold the 1/B mean scale into g; emit bf16 v directly
                g = small.tile([128, H, U], f32, tag="fg")
                nc.vector.scalar_tensor_tensor(
                    out=g, in0=sqrtm, scalar=1.0 / B, in1=rec,
                    op0=AL.mult, op1=AL.mult)
                vb16 = small.tile([128, H, N, U], bf16, tag="fvb16")
                nc.vector.tensor_tensor(
                    out=vb16, in0=s_sc,
                    in1=g.unsqueeze(2).broadcast_to([128, H, N, U]),
                    op=AL.mult)

                # ------------ full a-pass ------------
                # Per i: C matmuls (k = 2x128 accumulated over b) into a
                # 3-bank PSUM tile -> ONE ACT drain. The DVE product + u-fold
                # tree runs FUSED over i-pairs (0,1),(2,3),(4,5) to halve the
                # per-op dispatch overhead; i=6,7 stay single so the serial
                # tail chain after the last matmul is short. The i-reduction
                # over 0..5 is issued early, off the tail.
                z2 = small.tile([128, I, JT, N], bf16, tag="z2")
                for grp in ((0, 1), (2, 3), (4, 5), (6,), (7,)):
                    g = len(grp)
                    Cb = small.tile([128, g, JT, N, U], bf16, tag=f"Cb{g}")
                    for gi, i in enumerate(grp):
                        psc = ps_c.tile([128, 3, 512], f32, tag=f"c{i % 2}")
                        for jt in range(JT):
                            for h in range(H):
                                nc.tensor.matmul(
                                    psc[:, jt // 3,
                                        (jt % 3) * NU:(jt % 3 + 1) * NU],
                                    lhsT=XNf[:, i, h,
                                             jt * 128:(jt + 1) * 128],
                                    rhs=vb16[:, h, :, :],
                                    start=(h == 0), stop=(h == H - 1))
                        nc.scalar.copy(
                            out=Cb[:, gi].rearrange("p jt n u -> p (jt n u)"),
                            in_=psc[:, :, 0:3 * NU])
                    i0 = grp[0]
                    zt = small.tile([128, g, JT, N, U], bf16, tag=f"zt{g}")
                    nc.vector.tensor_tensor(
                        out=zt.rearrange("p g jt n u -> p (g jt n u)"),
                        in0=Wb[:, i0:i0 + g].rearrange(
                            "p g jt n u -> p (g jt n u)"),
                        in1=Cb.rearrange("p g jt n u -> p (g jt n u)"),
                        op=AL.mult)
                    t8 = small.tile([128, g, JT, N, 8], bf16, tag=f"t8{g}")
                    nc.vector.tensor_tensor(out=t8, in0=zt[:, :, :, :, 0:8],
                                            in1=zt[:, :, :, :, 8:16],
                                            op=AL.add)
                    t4 = small.tile([128, g, JT, N, 4], bf16, tag=f"t4{g}")
                    nc.vector.tensor_tensor(out=t4, in0=t8[:, :, :, :, 0:4],
                                            in1=t8[:, :, :, :, 4:8],
                                            op=AL.add)
                    t2 = small.tile([128, g, JT, N, 2], bf16, tag=f"t2{g}")
                    nc.vector.tensor_tensor(out=t2, in0=t4[:, :, :, :, 0:2],
                                            in1=t4[:, :, :, :, 2:4],
                                            op=AL.add)
                    nc.vector.tensor_tensor(out=z2[:, i0:i0 + g, :, :],
                                            in0=t2[:, :, :, :, 0],
                                            in1=t2[:, :, :, :, 1], op=AL.add)
                    if grp == (4, 5):
                        # fold i=0..5 of the i-reduction off the tail; on
                        # GpSimd so it doesn't lengthen the DVE fold stream
                        alo = small.tile([128, JT * N], bf16, tag="alo")
                        with nc.allow_low_precision(
                                reason="agreement accumulate; bf16 rounding "
                                       "is within tolerance"):
                            nc.vector.tensor_reduce(
                                out=alo,
                                in_=z2[:, 0:6].rearrange(
                                    "p i jt n -> p (jt n) i"),
                                axis=AX.X, op=AL.add)
                a67 = small.tile([128, JT * N], bf16, tag="a67")
                nc.vector.tensor_tensor(
                    out=a67,
                    in0=z2[:, 6].rearrange("p jt n -> p (jt n)"),
                    in1=z2[:, 7].rearrange("p jt n -> p (jt n)"), op=AL.add)
                apart = small.tile([128, JT * N], bf16, tag="apart")
                nc.vector.tensor_tensor(out=apart, in0=alo, in1=a67,
                                        op=AL.add)
                nc.vector.tensor_tensor(out=bmat, in0=bmat, in1=apart,
                                        op=AL.add)

    nc.compile()
    return nc


def _prep_inputs(x_full, W):
    """Host-side relayout. x_full: [B, I, J] f32, W: [J, N, U, I] f32."""
    import ml_dtypes
    bf = ml_dtypes.bfloat16
    # Wb[p, i, jt, n, u] = W[128*jt+p, n, u, i]
    Wb = np.ascontiguousarray(
        W.reshape(JT, 128, N, U, I).transpose(1, 4, 0, 2, 3)
    ).reshape(128, I * JT * N * U).astype(bf)
    f8 = ml_dtypes.float8_e4m3fn
    # x2f[p, i, jt, h, c] = x[128h+c, i, 128jt+p]
    X2F = np.ascontiguousarray(
        x_full.reshape(H, 128, I, JT, 128).transpose(4, 2, 3, 0, 1)
    ).reshape(128, I * JT * H * 128).astype(f8)
    # xnf[c, i, h, j] = x[128h+c, i, j]
    XNF = np.ascontiguousarray(
        x_full.reshape(H, 128, I, J).transpose(1, 2, 0, 3)
    ).reshape(128, I * H * J).astype(f8)
    in_maps = []
    for c in range(NCORES):
        xc = x_full[c * BL:(c + 1) * BL]                   # [32, 8, 1152]
        # x128[4b+ih, il, j] = xc[b, 2*ih+il, j]
        x128 = xc.reshape(BL, 4, 2, J).reshape(128, 2, J)
        # X2[p, t=(il*9+jt), c] = x128[c, il, 128*jt+p]
        X2 = np.ascontiguousarray(
            x128.reshape(128, 2, JT, 128).transpose(3, 1, 2, 0)
        ).reshape(128, 18 * 128).astype(bf)
        in_maps.append({"wb": Wb, "x2f": X2F, "xnf": XNF, "x2": X2})
    return in_maps


def kernel(x, W):
    """x: [256, 8, 1152] f32; W: [1152, 10, 16, 8] f32 ->
    v: [256, 10, 16, 1] f32."""
    from concourse.bass_utils import run_bass_kernel_spmd

    x = np.asarray(x, dtype=np.float32)
    W = np.asarray(W, dtype=np.float32)
    if "nc" not in _CACHE:
        _CACHE["nc"] = _build_nc()
    nc = _CACHE["nc"]
    in_maps = _prep_inputs(x, W)
    res = run_bass_kernel_spmd(nc, in_maps, core_ids=list(range(NCORES)))
    out = np.concatenate([r["v"] for r in res.results], axis=0)
    return out.reshape(B, N, U, 1).astype(np.float32)


if __name__ == "__main__":
    rng = np.random.default_rng(0)
    x = rng.standard_normal((B, I, J), dtype=np.float32)
    W = rng.standard_normal((J, N, U, I), dtype=np.float32)
    got = kernel(x, W)
    # numpy reference for a self-contained smoke test
    u_hat = np.einsum('jnui,bij->bjnu', W, x)
    b = np.zeros((J, N), dtype=np.float32)
    for _ in range(ITERS):
        e = np.exp(b - b.max(axis=0, keepdims=True))
        c = e / e.sum(axis=0, keepdims=True)
        s = np.einsum('jn,bjnu->bnu', c, u_hat)
        mag = np.sum(s * s, axis=1, keepdims=True)
        v = (mag / (1.0 + mag)) * (s / np.sqrt(mag))
        b = b + np.einsum('bjnu,bnu->jn', u_hat, v) / B
    exp = v[..., None]
    rel = np.linalg.norm(got - exp) / np.linalg.norm(exp)
    print("rel_fro:", rel)


# revision 43
# speedup vs baseline: 1.0376x; 1.0200x over previous
"""Trainium2 Bass kernel for the CapsuleLayer dynamic-routing module.

Strategy (8 NeuronCores, zero collectives):
  The [J,N] agreement update needs a mean over the full batch. Instead of
  data-parallel + AllReduce (first-execution ncfw staging costs ~25-45us
  per collective op and gates the routing loop), every core redundantly
  computes the FULL-batch s-pass and a-pass for routing iterations 0 and
  1, so the b_ij update is computed locally and identically on all cores.
  Only the last iteration's s-pass/squash runs on the core's own 32-row
  batch slice to produce its output shard. Extra cost: 2x matmul work on
  iters 0/1 plus ~9.4 MB of replicated x layouts streamed during iter-0
  compute. No inter-core sync at all.

  Host pre-lays-out (not part of measured HW time):
      wb   [128, (i=8, jt=9, n=10, u=16)]  bf16  -- W[j,n,u,i], j = 128*jt + p
      x2f  [128, (i, jt, h=2, c=128)]      fp8   -- x[b,i,j]: b = 128h + c
      xnf  [128, (i, h=2, j=1152)]         fp8   -- partitions = b % 128
      x2   [128, (t=18, c=128)]            bf16  -- core-local slice,
                                                    t=(i%2)*9+jt, c=4b+i//2
  The x layouts that feed only the b_ij update path are fp8e4m3 (mixed
  fp8-weights x bf16-moving matmuls): x-side quantization noise averages
  out over the batch/unit contractions (simulated rel-err 3.2e-3 ->
  4.0e-3), while quantizing v or the final-iteration x would break
  tolerance, so those stay bf16.

  Per routing iteration (full-batch, iters 0/1):
      s-pass:  144 accumulated PE matmuls (k=128 j-partitions, two b-halves
               into one PSUM tile [128, 2, 160]); lhsT = x2f slices,
               rhs = A slices (A = W*c).
      squash on [128, 2, (n,u)] with exact reference semantics; 1/B mean
               scale folded into the bf16 v used by the a-pass.
      a-pass:  C[(i,j), nu] = x^T v (PE, k=2x128 accumulated) into 3-bank
               PSUM ping-pong tiles, one ACT drain per i; z = Wb.*C and the
               pairwise u-fold tree run on DVE fused over i-pairs (0,1),
               (2,3),(4,5) to halve dispatch overhead, singly for i=6,7 so
               the serial tail after the last matmul stays short; the
               i-reduction over 0..5 is issued early, off the tail.
      c-pass:  cexp = exp(b); D[n] via ones-matmul; A_i = Wb_i * cexp_u
               (A_0 via a broadcast AP so the next s-pass starts sooner).
  Last iteration: 72 local matmuls (lhsT = core's bf16 x2 slices), local
  squash, DMA v [32, 160] f32 out.
"""

import numpy as np

B, I, J, N, U = 256, 8, 1152, 10, 16
NU = N * U            # 160
ITERS = 3
NCORES = 8
BL = B // NCORES      # 32
JT = 9                # 1152 / 128
H = 2                 # 256 / 128 batch halves

_CACHE = {}


def _build_nc():
    import concourse.bass as bass
    import concourse.bacc as bacc
    import concourse.tile as tile
    from concourse import mybir

    f32 = mybir.dt.float32
    bf16 = mybir.dt.bfloat16
    fp8 = mybir.dt.float8e4
    AL = mybir.AluOpType
    AF = mybir.ActivationFunctionType
    AX = mybir.AxisListType

    nc = bacc.Bacc("TRN2", target_bir_lowering=False, debug=False,
                   num_devices=NCORES)
    wb_d = nc.dram_tensor("wb", [128, I * JT * NU], bf16, kind="ExternalInput").ap()
    x2f_d = nc.dram_tensor("x2f", [128, I * JT * H * 128], fp8,
                           kind="ExternalInput").ap()
    xnf_d = nc.dram_tensor("xnf", [128, I * H * J], fp8,
                           kind="ExternalInput").ap()
    x2_d = nc.dram_tensor("x2", [128, 18 * 128], bf16, kind="ExternalInput").ap()
    v_d = nc.dram_tensor("v", [BL, NU], f32, kind="ExternalOutput").ap()

    with tile.TileContext(nc) as tc:
        with (
            tc.tile_pool(name="big", bufs=1) as big,
            tc.tile_pool(name="abp", bufs=1) as abp,
            tc.tile_pool(name="small", bufs=2) as small,
            tc.tile_pool(name="pers", bufs=1) as pers,
            tc.tile_pool(name="ps_s", bufs=1, space="PSUM") as ps_s,
            tc.tile_pool(name="ps_c", bufs=1, space="PSUM") as ps_c,
        ):
            # ---------------- load inputs ----------------
            # Interleave x2f_i / Wb_i per i so the iter-0 s-pass can start
            # as soon as the first (x2f_0, Wb_0) pair lands.
            x2f_v = x2f_d.rearrange("p (i jt h c) -> p i jt h c",
                                    i=I, jt=JT, h=H)
            wb_v = wb_d.rearrange("p (i jt n u) -> p i jt n u",
                                  i=I, jt=JT, n=N, u=U)
            # fp8 halves the x2f HBM traffic and speeds the 128-col weight
            # loads; the s-pass runs mixed fp8 lhsT x bf16 rhs. Simulated
            # end-to-end rel-err impact: 3.2e-3 -> 3.4e-3.
            X2f = big.tile([128, I, JT, H, 128], fp8)
            Wb = big.tile([128, I, JT, N, U], bf16)
            for i in range(I):
                nc.sync.dma_start(out=X2f[:, i], in_=x2f_v[:, i])
                nc.sync.dma_start(out=Wb[:, i], in_=wb_v[:, i])
            # fp8 x for the a-pass too (C matmuls run mixed fp8 x bf16 v);
            # only quantizing v breaks tolerance, x-side noise averages out.
            xnf_v = xnf_d.rearrange("p (i h j) -> p i h j", i=I, h=H)
            XNf = big.tile([128, I, H, J], fp8)
            for i in range(I):
                nc.sync.dma_start(out=XNf[:, i], in_=xnf_v[:, i])
            X2 = big.tile([128, 18, 128], bf16)
            nc.sync.dma_start(out=X2, in_=x2_d.rearrange(
                "p (t c) -> p t c", t=18))

            ones = pers.tile([128, 128], bf16)
            nc.vector.memset(ones, 1.0)
            bmat = pers.tile([128, JT * N], f32)          # b[j, n] as [p, (jt, n)]
            nc.vector.memset(bmat, 0.0)

            X2v = X2.rearrange("p t (b ih) -> p t ih b", ih=4)

            for it in range(ITERS):
                first = it == 0
                last = it == ITERS - 1

                # ------------ c-pass: A and Dinv ------------
                if first:
                    As = Wb
                    Dinv = small.tile([128, N], f32, tag="dinv")
                    nc.vector.memset(Dinv, 1.0 / J)
                else:
                    cexp_b = small.tile([128, JT * N], bf16, tag="cexpb")
                    nc.scalar.activation(out=cexp_b, in_=bmat, func=AF.Exp)
                    # D[n] = sum_{p,jt} cexp -> ones-matmul (replicated over
                    # all 128 partitions) + jt-reduce
                    psd = ps_c.tile([128, 3, 512], f32, tag="c0")
                    nc.tensor.matmul(psd[:, 0, 0:JT * N], lhsT=ones,
                                     rhs=cexp_b, start=True, stop=True)
                    D32 = small.tile([128, N], f32, tag="d32")
                    nc.vector.tensor_reduce(
                        out=D32,
                        in_=psd[:, 0, 0:JT * N].rearrange(
                            "q (jt n) -> q n jt", jt=JT),
                        axis=AX.X, op=AL.add)
                    Dinv = small.tile([128, N], f32, tag="dinv")
                    nc.vector.reciprocal(out=Dinv, in_=D32)
                    As = abp.tile([128, I, JT, N, U], bf16, tag="Aall")
                    # A_0 reads cexp via a broadcast AP so the first s-pass
                    # matmuls don't wait for the cexp_u materialization.
                    nc.vector.tensor_tensor(
                        out=As[:, 0],
                        in0=Wb[:, 0],
                        in1=cexp_b.rearrange("p (jt n) -> p jt n", jt=JT)
                        .unsqueeze(3).broadcast_to([128, JT, N, U]),
                        op=AL.mult)
                    # cexp_u[p, jt, n, u] = cexp_b[p, jt, n]  (broadcast u)
                    cexp_u = small.tile([128, JT, N, U], bf16, tag="cexpu")
                    nc.vector.tensor_copy(
                        out=cexp_u,
                        in_=cexp_b.rearrange("p (jt n) -> p jt n", jt=JT)
                        .unsqueeze(3).broadcast_to([128, JT, N, U]))
                    cexp_flat = cexp_u.rearrange("p jt n u -> p (jt n u)")
                    for i in range(1, I):
                        nc.vector.tensor_tensor(
                            out=As[:, i].rearrange("p jt n u -> p (jt n u)"),
                            in0=Wb[:, i].rearrange("p jt n u -> p (jt n u)"),
                            in1=cexp_flat, op=AL.mult)

                if last:
                    # ------------ local s-pass: 72 matmuls ------------
                    pss = ps_s.tile([BL, NU], f32, tag="pss")
                    k = 0
                    for i in range(I):
                        il, ih = i % 2, i // 2
                        for jt in range(JT):
                            nc.tensor.matmul(
                                pss,
                                lhsT=X2v[:, il * JT + jt, ih, :],
                                rhs=As[:, i, jt, :, :],
                                start=(k == 0), stop=(k == 71))
                            k += 1
                    # ------------ local squash + output ------------
                    s_sc = small.tile([BL, N, U], f32, tag="ssc")
                    nc.vector.tensor_tensor(
                        out=s_sc,
                        in0=pss.rearrange("b (n u) -> b n u", n=N),
                        in1=Dinv[0:BL].unsqueeze(2).broadcast_to([BL, N, U]),
                        op=AL.mult)
                    sq = small.tile([BL, N, U], f32, tag="sq")
                    nc.vector.tensor_tensor(out=sq, in0=s_sc, in1=s_sc,
                                            op=AL.mult)
                    mag = small.tile([BL, U], f32, tag="mag")
                    nc.vector.tensor_reduce(
                        out=mag, in_=sq.rearrange("b n u -> b u n"),
                        axis=AX.X, op=AL.add)
                    sqrtm = small.tile([BL, U], f32, tag="sqrtm")
                    nc.scalar.activation(out=sqrtm, in_=mag, func=AF.Sqrt)
                    onep = small.tile([BL, U], f32, tag="onep")
                    nc.vector.tensor_scalar_add(out=onep, in0=mag, scalar1=1.0)
                    rec = small.tile([BL, U], f32, tag="rec")
                    nc.vector.reciprocal(out=rec, in_=onep)
                    g = small.tile([BL, U], f32, tag="g")
                    nc.vector.tensor_tensor(out=g, in0=sqrtm, in1=rec,
                                            op=AL.mult)
                    v_f32 = small.tile([BL, N, U], f32, tag="vf32")
                    nc.vector.tensor_tensor(
                        out=v_f32, in0=s_sc,
                        in1=g.unsqueeze(1).broadcast_to([BL, N, U]),
                        op=AL.mult)
                    nc.sync.dma_start(
                        out=v_d, in_=v_f32.rearrange("b n u -> b (n u)"))
                    break

                # ------------ full s-pass: 144 accumulated matmuls ------------
                PSS = ps_s.tile([128, H, NU], f32)
                for i in range(I):
                    for jt in range(JT):
                        for h in range(H):
                            nc.tensor.matmul(
                                PSS[:, h, :],
                                lhsT=X2f[:, i, jt, h, :],
                                rhs=As[:, i, jt, :, :],
                                start=(i == 0 and jt == 0),
                                stop=(i == I - 1 and jt == JT - 1))

                # ------------ full squash ------------
                s_sc = small.tile([128, H, N, U], f32, tag="fssc")
                nc.vector.tensor_tensor(
                    out=s_sc,
                    in0=PSS.rearrange("q h (n u) -> q h n u", n=N),
                    in1=Dinv.unsqueeze(1).unsqueeze(3)
                        .broadcast_to([128, H, N, U]),
                    op=AL.mult)
                sq = small.tile([128, H, N, U], f32, tag="fsq")
                nc.vector.tensor_tensor(out=sq, in0=s_sc, in1=s_sc,
                                        op=AL.mult)
                mag = small.tile([128, H, U], f32, tag="fmag")
                nc.vector.tensor_reduce(
                    out=mag, in_=sq.rearrange("q h n u -> q h u n"),
                    axis=AX.X, op=AL.add)
                sqrtm = small.tile([128, H, U], f32, tag="fsqrtm")
                nc.scalar.activation(out=sqrtm, in_=mag, func=AF.Sqrt)
                onep = small.tile([128, H, U], f32, tag="fonep")
                nc.vector.tensor_scalar_add(out=onep, in0=mag, scalar1=1.0)
                rec = small.tile([128, H, U], f32, tag="frec")
                nc.vector.reciprocal(out=rec, in_=onep)
                # fold the 1/B mean scale into g; emit bf16 v directly
                g = small.tile([128, H, U], f32, tag="fg")
                nc.vector.scalar_tensor_tensor(
                    out=g, in0=sqrtm, scalar=1.0 / B, in1=rec,
                    op0=AL.mult, op1=AL.mult)
                vb16 = small.tile([128, H, N, U], bf16, tag="fvb16")
                nc.vector.tensor_tensor(
                    out=vb16, in0=s_sc,
                    in1=g.unsqueeze(2).broadcast_to([128, H, N, U]),
                    op=AL.mult)

                # ------------ full a-pass ------------
                # Per i: C matmuls (k = 2x128 accumulated over b) into a
                # 3-bank PSUM tile -> ONE ACT drain. The DVE product + u-fold
                # tree runs FUSED over i-pairs (0,1),(2,3),(4,5) to halve the
                # per-op dispatch overhead; i=6,7 stay single so the serial
                # tail chain after the last matmul is short. The i-reduction
                # over 0..5 is issued early, off the tail.
                z2 = small.tile([128, I, JT, N], bf16, tag="z2")
                for grp in ((0, 1), (2, 3), (4, 5), (6,), (7,)):
                    g = len(grp)
                    Cb = small.tile([128, g, JT, N, U], bf16, tag=f"Cb{g}")
                    for gi, i in enumerate(grp):
                        psc = ps_c.tile([128, 3, 512], f32, tag=f"c{i % 2}")
                        for jt in range(JT):
                            for h in range(H):
                                nc.tensor.matmul(
                                    psc[:, jt // 3,
                                        (jt % 3) * NU:(jt % 3 + 1) * NU],
                                    lhsT=XNf[:, i, h,
                                             jt * 128:(jt + 1) * 128],
                                    rhs=vb16[:, h, :, :],
                                    start=(h == 0), stop=(h == H - 1))
                        nc.scalar.copy(
                            out=Cb[:, gi].rearrange("p jt n u -> p (jt n u)"),
                            in_=psc[:, :, 0:3 * NU])
                    i0 = grp[0]
                    zt = small.tile([128, g, JT, N, U], bf16, tag=f"zt{g}")
                    nc.vector.tensor_tensor(
                        out=zt.rearrange("p g jt n u -> p (g jt n u)"),
                        in0=Wb[:, i0:i0 + g].rearrange(
                            "p g jt n u -> p (g jt n u)"),
                        in1=Cb.rearrange("p g jt n u -> p (g jt n u)"),
                        op=AL.mult)
                    t8 = small.tile([128, g, JT, N, 8], bf16, tag=f"t8{g}")
                    nc.vector.tensor_tensor(out=t8, in0=zt[:, :, :, :, 0:8],
                                            in1=zt[:, :, :, :, 8:16],
                                            op=AL.add)
                    t4 = small.tile([128, g, JT, N, 4], bf16, tag=f"t4{g}")
                    nc.vector.tensor_tensor(out=t4, in0=t8[:, :, :, :, 0:4],
                                            in1=t8[:, :, :, :, 4:8],
                                            op=AL.add)
                    t2 = small.tile([128, g, JT, N, 2], bf16, tag=f"t2{g}")
                    nc.vector.tensor_tensor(out=t2, in0=t4[:, :, :, :, 0:2],
                                            in1=t4[:, :, :, :, 2:4],
                                            op=AL.add)
                    nc.vector.tensor_tensor(out=z2[:, i0:i0 + g, :, :],
                                            in0=t2[:, :, :, :, 0],
                                            in1=t2[:, :, :, :, 1], op=AL.add)
                    if grp == (4, 5):
                        # fold i=0..5 of the i-reduction off the tail; on
                        # GpSimd so it doesn't lengthen the DVE fold stream
                        alo = small.tile([128, JT * N], bf16, tag="alo")
                        with nc.allow_low_precision(
                                reason="agreement accumulate; bf16 rounding "
                                       "is within tolerance"):
                            nc.vector.tensor_reduce(
                                out=alo,
                                in_=z2[:, 0:6].rearrange(
                                    "p i jt n -> p (jt n) i"),
                                axis=AX.X, op=AL.add)
                a67 = small.tile([128, JT * N], bf16, tag="a67")
                nc.vector.tensor_tensor(
                    out=a67,
                    in0=z2[:, 6].rearrange("p jt n -> p (jt n)"),
                    in1=z2[:, 7].rearrange("p jt n -> p (jt n)"), op=AL.add)
                apart = small.tile([128, JT * N], bf16, tag="apart")
                nc.vector.tensor_tensor(out=apart, in0=alo, in1=a67,
                                        op=AL.add)
                nc.vector.tensor_tensor(out=bmat, in0=bmat, in1=apart,
                                        op=AL.add)

    nc.compile()
    return nc


def _prep_inputs(x_full, W):
    """Host-side relayout. x_full: [B, I, J] f32, W: [J, N, U, I] f32."""
    import ml_dtypes
    bf = ml_dtypes.bfloat16
    # Wb[p, i, jt, n, u] = W[128*jt+p, n, u, i]
    Wb = np.ascontiguousarray(
        W.reshape(JT, 128, N, U, I).transpose(1, 4, 0, 2, 3)
    ).reshape(128, I * JT * N * U).astype(bf)
    f8 = ml_dtypes.float8_e4m3fn
    # x2f[p, i, jt, h, c] = x[128h+c, i, 128jt+p]
    X2F = np.ascontiguousarray(
        x_full.reshape(H, 128, I, JT, 128).transpose(4, 2, 3, 0, 1)
    ).reshape(128, I * JT * H * 128).astype(f8)
    # xnf[c, i, h, j] = x[128h+c, i, j]
    XNF = np.ascontiguousarray(
        x_full.reshape(H, 128, I, J).transpose(1, 2, 0, 3)
    ).reshape(128, I * H * J).astype(f8)
    in_maps = []
    for c in range(NCORES):
        xc = x_full[c * BL:(c + 1) * BL]                   # [32, 8, 1152]
        # x128[4b+ih, il, j] = xc[b, 2*ih+il, j]
        x128 = xc.reshape(BL, 4, 2, J).reshape(128, 2, J)
        # X2[p, t=(il*9+jt), c] = x128[c, il, 128*jt+p]
        X2 = np.ascontiguousarray(
            x128.reshape(128, 2, JT, 128).transpose(3, 1, 2, 0)
        ).reshape(128, 18 * 128).astype(bf)
        in_maps.append({"wb": Wb, "x2f": X2F, "xnf": XNF, "x2": X2})
    return in_maps


def kernel(x, W):
    """x: [256, 8, 1152] f32; W: [1152, 10, 16, 8] f32 ->
    v: [256, 10, 16, 1] f32."""
    from concourse.bass_utils import run_bass_kernel_spmd

    x = np.asarray(x, dtype=np.float32)
    W = np.asarray(W, dtype=np.float32)
    if "nc" not in _CACHE:
        _CACHE["nc"] = _build_nc()
    nc = _CACHE["nc"]
    in_maps = _prep_inputs(x, W)
    res = run_bass_kernel_spmd(nc, in_maps, core_ids=list(range(NCORES)))
    out = np.concatenate([r["v"] for r in res.results], axis=0)
    return out.reshape(B, N, U, 1).astype(np.float32)


if __name__ == "__main__":
    rng = np.random.default_rng(0)
    x = rng.standard_normal((B, I, J), dtype=np.float32)
    W = rng.standard_normal((J, N, U, I), dtype=np.float32)
    got = kernel(x, W)
    # numpy reference for a self-contained smoke test
    u_hat = np.einsum('jnui,bij->bjnu', W, x)
    b = np.zeros((J, N), dtype=np.float32)
    for _ in range(ITERS):
        e = np.exp(b - b.max(axis=0, keepdims=True))
        c = e / e.sum(axis=0, keepdims=True)
        s = np.einsum('jn,bjnu->bnu', c, u_hat)
        mag = np.sum(s * s, axis=1, keepdims=True)
        v = (mag / (1.0 + mag)) * (s / np.sqrt(mag))
        b = b + np.einsum('bjnu,bnu->jn', u_hat, v) / B
    exp = v[..., None]
    rel = np.linalg.norm(got - exp) / np.linalg.norm(exp)
    print("rel_fro:", rel)
